# revision 20
# baseline (speedup 1.0000x reference)
"""Causal multi-head attention on 8 TRN2 NeuronCores.

Sharding: tensor-parallel over heads. Each core owns 2 of the 16 heads:
column slices of Wq/Wk/Wv, row slice of Wo. The final output-projection
partials are summed with chunked ReduceScatters (token-sharded, overlapped
with compute), bias added on chip, host reassembles the shards.

Shapes (hardcoded): B=2, S=2048, D=1024, H=16, HD=64.

All PE-facing tensors are fp16 (cast on host / on copy); PSUM accumulation
and softmax denominators stay fp32; the cross-core reduction runs fp16.

Per-core dataflow:
  A) xT tiles via DMA-transpose (fp16, HWDGE xbar); QT/KT = W_c.T @ xT
     (N=512); V = xT.T @ Wv_c directly in natural [tok, feat] layout,
     stored per (batch, k-tile) as [128, 65] = [V_head | ones-column].
  B) per (batch, 512-query-chunk, head): scores^T[k,q] = KT.T @ QT
     (K=64 contraction), additive causal mask on diagonal 128x128 blocks,
     exp on ACT (scale=1/8, no max subtraction: |scores| <~ 3), then
     ctx^T[d,q] accumulated over k-tiles with lhsT=[V|1] so row 64 is the
     softmax denominator. Normalize via batched reciprocal +
     partition-broadcast.
  C) fused per query-chunk: out partial [q,1024] = ctxT(2 heads, K=128).T
     @ Wo_c -> cc_in rows; ReduceScatter(add) per 512-token chunk
     (overlaps with the next chunk's attention); +bias -> out shard.
"""

import numpy as np

import concourse.bacc as bacc
import concourse.bass as bass
import concourse.mybir as mybir
from concourse.bass_utils import run_bass_kernel_spmd
from concourse.masks import make_identity
from concourse.tile import TileContext

B, S, D, H = 2, 2048, 1024, 16
HD = D // H            # 64
NCORES = 8
HPC = H // NCORES      # 2 heads per core
FPC = HPC * HD         # 128 feature cols per core
T = B * S              # 4096 tokens
CHUNK = 512            # token chunk for stage A
QC = 512               # query chunk for stage B / RS chunk
NCH = T // QC          # 8 chunks
SHARD = T // NCORES    # 512 rows per core output (8 chunks x 64 rows)
KT = 128               # k-tile size
NKT = S // KT          # 16 k-tiles per batch
F32 = mybir.dt.float32
F16 = mybir.dt.float16
MASK_NEG = -240.0      # exp((s-240)/8) ~ exp(-30) ~ 1e-13


def build_nc():
    nc = bacc.Bacc(num_devices=NCORES)

    x_d = nc.dram_tensor("x", [T, D], F16, kind="ExternalInput")
    wq_d = nc.dram_tensor("wq", [D, FPC], F16, kind="ExternalInput")
    wk_d = nc.dram_tensor("wk", [D, FPC], F16, kind="ExternalInput")
    wv_d = nc.dram_tensor("wv", [D, FPC], F16, kind="ExternalInput")
    wo_d = nc.dram_tensor("wo", [FPC, D], F16, kind="ExternalInput")
    bo_d = nc.dram_tensor("bo", [1, D], F32, kind="ExternalInput")
    cc_warm_in = nc.dram_tensor("cc_warm_in", [NCORES, 16], F32, kind="Internal")
    cc_warm_out = nc.dram_tensor("cc_warm_out", [1, 16], F32, kind="Internal")
    cc_in = [nc.dram_tensor(f"cc_in{c}", [QC, D], F16, kind="Internal")
             for c in range(NCH)]
    cc_out = [nc.dram_tensor(f"cc_out{c}", [QC // NCORES, D], F16, kind="Internal")
              for c in range(NCH)]
    out_d = nc.dram_tensor("out", [SHARD, D], F32, kind="ExternalOutput")

    with TileContext(nc) as tc:
        with (
            tc.tile_pool(name="const", bufs=1) as constp,
            tc.tile_pool(name="wts", bufs=1) as wp,
            tc.tile_pool(name="big", bufs=1) as bigp,
        ):
            # --- constants ---
            # causal additive mask for diagonal blocks of scores^T[k, q]:
            # keep 0 where (q - k) >= 0, else MASK_NEG.
            cmask = constp.tile([128, 128], F32)
            nc.gpsimd.memset(cmask, 0.0)
            nc.gpsimd.affine_select(
                out=cmask,
                in_=cmask,
                compare_op=mybir.AluOpType.is_ge,
                fill=MASK_NEG,
                base=0,
                pattern=[[1, 128]],
                channel_multiplier=-1,
            )
            nc.gpsimd.collective_compute(
                "ReduceScatter",
                mybir.AluOpType.add,
                replica_groups=[list(range(NCORES))],
                ins=[cc_warm_in[:, :]],
                outs=[cc_warm_out[:, :]],
            )
            bo_row = constp.tile([1, D], F32)
            nc.scalar.dma_start(bo_row, bo_d[0:1, :])
            bo_bc = constp.tile([128, D], F32)
            nc.gpsimd.partition_broadcast(bo_bc, bo_row)

            # --- weights ---
            wq_sb = wp.tile([128, 8, FPC], F16)
            wk_sb = wp.tile([128, 8, FPC], F16)
            wv_sb = wp.tile([128, 8, FPC], F16)
            for w_sb, w_dram in ((wq_sb, wq_d), (wk_sb, wk_d), (wv_sb, wv_d)):
                for j in range(8):
                    nc.scalar.dma_start(w_sb[:, j, :], w_dram[j * 128:(j + 1) * 128, :])
            wo_sb = wp.tile([128, D], F16)
            nc.scalar.dma_start(wo_sb, wo_d[:, :])

            # --- resident activations ---
            qt_sb = bigp.tile([128, T], F16)     # Q^T  [feat(2 heads x 64), tok]
            kt_sb = bigp.tile([128, T], F16)     # K^T
            ctxt_sb = bigp.tile([128, T], F16)   # normalized ctx^T (heads stacked)
            v_sb = bigp.tile([128, B, NKT, HPC * (HD + 1)], F16)  # [V_h|1] tiles
            ones_col = constp.tile([128, 1], F32)
            nc.gpsimd.memset(ones_col, 1.0)
            for h in range(HPC):
                c = h * (HD + 1) + HD
                nc.vector.tensor_copy(
                    v_sb[:, :, :, c:c + 1],
                    ones_col[:, None, None, :].broadcast_to([128, B, NKT, 1]),
                )

            # ---- interleaved stage A (projection) + stage B/C ----------
            # Engines execute their streams in emission order, so stage-A
            # work for later token spans is emitted between attention
            # chunks to keep every engine's stream in pipeline order.
            with (
                tc.tile_pool(name="xt", bufs=8) as xtp,
                tc.tile_pool(name="sbB", bufs=3) as sbB,
                tc.tile_pool(name="nrm", bufs=2) as nrm,
                tc.tile_pool(name="sbO", bufs=2) as sbO,
                tc.tile_pool(name="outp", bufs=2) as outp,
                tc.tile_pool(name="psA", bufs=1, space="PSUM") as psA,
                tc.tile_pool(name="psS", bufs=2, space="PSUM") as psS,
                tc.tile_pool(name="psC", bufs=2, space="PSUM") as psC,
                tc.tile_pool(name="psO", bufs=1, space="PSUM") as psO,
            ):
                def emit_a_dma(t0, w):
                    xt = xtp.tile([128, 8, CHUNK], F16, tag="xt")
                    if w <= 128:
                        for j in range(8):
                            nc.sync.dma_start_transpose(
                                xt[:, j, :w],
                                x_d[t0:t0 + w, j * 128:(j + 1) * 128],
                            )
                    else:
                        nc.sync.dma_start_transpose(
                            xt[:, :, :w], x_d[t0:t0 + w, :])
                    return xt

                def emit_a_proj(xt, t0, w):
                    for w_sb, dst in ((wq_sb, qt_sb), (wk_sb, kt_sb)):
                        pp = psA.tile([128, CHUNK], F32, tag="proj")
                        for j in range(8):
                            nc.tensor.matmul(
                                pp[:, :w], w_sb[:, j, :], xt[:, j, :w],
                                start=(j == 0), stop=(j == 7),
                            )
                        nc.vector.tensor_copy(dst[:, t0:t0 + w], pp[:, :w])
                    # V directly in natural [tok, feat] layout: xT.T @ Wv
                    b = t0 // S
                    for t in range(w // 128):
                        kt_idx = (t0 + t * 128 - b * S) // KT
                        pv = psA.tile([128, FPC], F32, tag="pv")
                        for j in range(8):
                            nc.tensor.matmul(
                                pv, xt[:, j, t * 128:(t + 1) * 128], wv_sb[:, j, :],
                                start=(j == 0), stop=(j == 7),
                            )
                        for h in range(HPC):
                            nc.vector.tensor_copy(
                                v_sb[:, b, kt_idx, h * (HD + 1):h * (HD + 1) + HD],
                                pv[:, h * HD:(h + 1) * HD],
                            )

                def emit_a(t0, w):
                    emit_a_proj(emit_a_dma(t0, w), t0, w)

                def emit_attn(ch):
                    b, qc = ch // (S // QC), ch % (S // QC)
                    for h in range(HPC):
                        pc = psC.tile([HD + 1, QC], F32, tag="ctx")
                        n_kt = (qc + 1) * (QC // KT)
                        for kt in range(n_kt):
                            diag = kt - qc * (QC // KT)
                            col_off = max(0, diag * KT)
                            n = QC - col_off
                            ps = psS.tile([128, QC], F32, tag="s")
                            nc.tensor.matmul(
                                ps[:, :n],
                                kt_sb[h * HD:(h + 1) * HD,
                                      b * S + kt * KT:b * S + (kt + 1) * KT],
                                qt_sb[h * HD:(h + 1) * HD,
                                      b * S + qc * QC + col_off:
                                      b * S + (qc + 1) * QC],
                                start=True, stop=True,
                            )
                            if diag >= 0:
                                nc.vector.tensor_add(
                                    ps[:, 0:KT], ps[:, 0:KT], cmask)
                            ex = sbB.tile([128, QC], F16, tag="exp")
                            nc.scalar.activation(
                                ex[:, :n], ps[:, :n],
                                mybir.ActivationFunctionType.Exp,
                                scale=0.125,
                            )
                            nc.tensor.matmul(
                                pc[:, col_off:QC],
                                v_sb[:, b, kt, h * (HD + 1):(h + 1) * (HD + 1)],
                                ex[:, :n],
                                start=(kt == 0), stop=(kt == n_kt - 1),
                            )
                        rrow = nrm.tile([1, QC], F32, tag="rrow")
                        nc.vector.reciprocal(rrow, pc[HD:HD + 1, :])
                        rec64 = nrm.tile([HD, QC], F32, tag="rec64")
                        nc.gpsimd.partition_broadcast(rec64, rrow)
                        nc.vector.tensor_mul(
                            ctxt_sb[h * HD:(h + 1) * HD,
                                    b * S + qc * QC:b * S + (qc + 1) * QC],
                            pc[0:HD, :], rec64,
                        )
                    # output projection for this chunk's 4 query tiles
                    for qt in range(QC // 128):
                        po = psO.tile([128, D], F32, tag="o")
                        row0 = ch * QC + qt * 128
                        for n in range(2):
                            nc.tensor.matmul(
                                po[:, n * 512:(n + 1) * 512],
                                ctxt_sb[:, row0:row0 + 128],
                                wo_sb[:, n * 512:(n + 1) * 512],
                                start=True, stop=True,
                            )
                        so = sbO.tile([128, D], F16, tag="so")
                        if qt % 2 == 0:
                            nc.scalar.copy(so, po)
                        else:
                            nc.vector.tensor_copy(so, po)
                        nc.gpsimd.dma_start(cc_in[ch][qt * 128:(qt + 1) * 128, :], so)
                    # reduce-scatter this chunk; overlaps later compute.
                    # Rank r receives rows [ch*QC + r*64, +64).
                    nc.gpsimd.collective_compute(
                        "ReduceScatter",
                        mybir.AluOpType.add,
                        replica_groups=[list(range(NCORES))],
                        ins=[cc_in[ch][:, :]],
                        outs=[cc_out[ch][:, :]],
                    )

                def emit_out(ch):
                    ot16 = outp.tile([QC // NCORES, D], F16, tag="ot16")
                    nc.sync.dma_start(ot16, cc_out[ch][:, :])
                    ot32 = outp.tile([QC // NCORES, D], F32, tag="ot32")
                    nc.vector.tensor_add(ot32, ot16, bo_bc[0:QC // NCORES, :])
                    nc.sync.dma_start(
                        out_d[ch * (QC // NCORES):(ch + 1) * (QC // NCORES), :],
                        ot32)

                # batch 0 light-first with A interleaved; batch 1
                # heavy-first so the tail chunks are light. The b1 xT
                # transpose DMAs are prefetched between b0 chunks; the
                # RS-dependent output stage is emitted last so it never
                # blocks the DMA queue.
                spans = [(0, 128), (128, 128), (256, 256), (512, 512),
                         (1024, 512), (1536, 512), (2048, 512),
                         (2560, 512), (3072, 512), (3584, 512)]
                xts = [emit_a_dma(t0, w) for t0, w in spans]
                emit_a_proj(xts[0], *spans[0])
                emit_a_proj(xts[1], *spans[1])
                emit_a_proj(xts[2], *spans[2])
                emit_attn(0)
                emit_a_proj(xts[3], *spans[3])
                emit_attn(1)
                emit_a_proj(xts[4], *spans[4])
                emit_attn(2)
                emit_a_proj(xts[5], *spans[5])
                emit_attn(3)
                emit_a_proj(xts[6], *spans[6])
                emit_a_proj(xts[7], *spans[7])
                emit_a_proj(xts[8], *spans[8])
                emit_a_proj(xts[9], *spans[9])
                emit_attn(7)
                emit_attn(6)
                emit_attn(5)
                emit_attn(4)
                for ch in [0, 1, 2, 3, 7, 6, 5, 4]:
                    emit_out(ch)

    nc.finalize()
    return nc


_NC_CACHE = []


def make_in_maps(x, Wq, Wk, Wv, Wo, bo):
    x = np.ascontiguousarray(np.asarray(x, dtype=np.float32)).reshape(T, D)
    x16 = x.astype(np.float16)
    Wq = np.asarray(Wq, dtype=np.float32).astype(np.float16)
    Wk = np.asarray(Wk, dtype=np.float32).astype(np.float16)
    Wv = np.asarray(Wv, dtype=np.float32).astype(np.float16)
    Wo = np.asarray(Wo, dtype=np.float32).astype(np.float16)
    bo = np.asarray(bo, dtype=np.float32).reshape(1, D)
    in_maps = []
    for c in range(NCORES):
        lo, hi = c * FPC, (c + 1) * FPC
        in_maps.append({
            "x": x16,
            "wq": np.ascontiguousarray(Wq[:, lo:hi]),
            "wk": np.ascontiguousarray(Wk[:, lo:hi]),
            "wv": np.ascontiguousarray(Wv[:, lo:hi]),
            "wo": np.ascontiguousarray(Wo[lo:hi, :]),
            "bo": bo,
        })
    return in_maps


def assemble_out(core_outs):
    # core r, chunk ch rows [ch*64, +64) = tokens [ch*512 + r*64, +64)
    stacked = np.stack(
        [np.asarray(o).reshape(NCH, QC // NCORES, D) for o in core_outs], axis=1
    )  # [ch, rank, 64, D]
    return stacked.reshape(B, S, D)


def kernel(x, Wq, Wk, Wv, Wo, bo):
    if not _NC_CACHE:
        _NC_CACHE.append(build_nc())
    nc = _NC_CACHE[0]
    in_maps = make_in_maps(x, Wq, Wk, Wv, Wo, bo)
    res = run_bass_kernel_spmd(nc, in_maps, core_ids=list(range(NCORES)))
    return assemble_out([r["out"] for r in res.results])


# revision 21
# speedup vs baseline: 1.1162x; 1.1162x over previous
"""Causal multi-head attention on 8 TRN2 NeuronCores.

Sharding: tensor-parallel over heads. Each core owns 2 of the 16 heads:
column slices of Wq/Wk/Wv, row slice of Wo. The final output-projection
partials are summed with chunked ReduceScatters (token-sharded, overlapped
with compute), bias added on chip, host reassembles the shards.

Shapes (hardcoded): B=2, S=2048, D=1024, H=16, HD=64.

All PE-facing tensors are fp16 (cast on host / on copy); PSUM accumulation
and softmax denominators stay fp32; the cross-core reduction runs fp16.

Per-core dataflow:
  A) xT tiles via DMA-transpose (fp16, HWDGE xbar); QT/KT = W_c.T @ xT
     (N=512); V = xT.T @ Wv_c directly in natural [tok, feat] layout,
     stored per (batch, k-tile) as [128, 65] = [V_head | ones-column].
  B) per (batch, 512-query-chunk, head): scores^T[k,q] = KT.T @ QT
     (K=64 contraction), additive causal mask on diagonal 128x128 blocks,
     exp on ACT (scale=1/8, no max subtraction: |scores| <~ 3), then
     ctx^T[d,q] accumulated over k-tiles with lhsT=[V|1] so row 64 is the
     softmax denominator. Normalize via batched reciprocal +
     partition-broadcast.
  C) fused per query-chunk: out partial [q,1024] = ctxT(2 heads, K=128).T
     @ Wo_c -> cc_in rows; ReduceScatter(add) per 512-token chunk
     (overlaps with the next chunk's attention); +bias -> out shard.
"""

import numpy as np

import concourse.bacc as bacc
import concourse.bass as bass
import concourse.mybir as mybir
from concourse.bass_utils import run_bass_kernel_spmd
from concourse.masks import make_identity
from concourse.tile import TileContext

B, S, D, H = 2, 2048, 1024, 16
HD = D // H            # 64
NCORES = 8
HPC = H // NCORES      # 2 heads per core
FPC = HPC * HD         # 128 feature cols per core
T = B * S              # 4096 tokens
CHUNK = 512            # token chunk for stage A
QC = 512               # query chunk for stage B / RS chunk
NCH = T // QC          # 8 chunks
SHARD = T // NCORES    # 512 rows per core output (8 chunks x 64 rows)
KT = 128               # k-tile size
NKT = S // KT          # 16 k-tiles per batch
F32 = mybir.dt.float32
F16 = mybir.dt.float16
MASK_NEG = -240.0      # exp((s-240)/8) ~ exp(-30) ~ 1e-13


def build_nc():
    nc = bacc.Bacc(num_devices=NCORES)

    x_d = nc.dram_tensor("x", [T, D], F16, kind="ExternalInput")
    wq_d = nc.dram_tensor("wq", [D, FPC], F16, kind="ExternalInput")
    wk_d = nc.dram_tensor("wk", [D, FPC], F16, kind="ExternalInput")
    wv_d = nc.dram_tensor("wv", [D, FPC], F16, kind="ExternalInput")
    wo_d = nc.dram_tensor("wo", [FPC, D], F16, kind="ExternalInput")
    bo_d = nc.dram_tensor("bo", [1, D], F32, kind="ExternalInput")
    cc_warm_in = nc.dram_tensor("cc_warm_in", [NCORES, 16], F32, kind="Internal")
    cc_warm_out = nc.dram_tensor("cc_warm_out", [1, 16], F32, kind="Internal")
    cc_in = [nc.dram_tensor(f"cc_in{c}", [QC, D], F16, kind="Internal")
             for c in range(NCH)]
    cc_out = [nc.dram_tensor(f"cc_out{c}", [QC // NCORES, D], F16, kind="Internal")
              for c in range(NCH)]
    out_d = nc.dram_tensor("out", [SHARD, D], F32, kind="ExternalOutput")

    with TileContext(nc) as tc:
        with (
            tc.tile_pool(name="const", bufs=1) as constp,
            tc.tile_pool(name="wts", bufs=1) as wp,
            tc.tile_pool(name="big", bufs=1) as bigp,
        ):
            # --- constants ---
            # causal additive mask for diagonal blocks of scores^T[k, q]:
            # keep 0 where (q - k) >= 0, else MASK_NEG.
            cmask = constp.tile([128, 128], F32)
            nc.gpsimd.memset(cmask, 0.0)
            nc.gpsimd.affine_select(
                out=cmask,
                in_=cmask,
                compare_op=mybir.AluOpType.is_ge,
                fill=MASK_NEG,
                base=0,
                pattern=[[1, 128]],
                channel_multiplier=-1,
            )
            bo_row = constp.tile([1, D], F32)
            nc.scalar.dma_start(bo_row, bo_d[0:1, :])
            bo_bc = constp.tile([128, D], F32)
            nc.gpsimd.partition_broadcast(bo_bc, bo_row)

            # --- weights ---
            wq_sb = wp.tile([128, 8, FPC], F16)
            wk_sb = wp.tile([128, 8, FPC], F16)
            wv_sb = wp.tile([128, 8, FPC], F16)
            for w_sb, w_dram in ((wq_sb, wq_d), (wk_sb, wk_d), (wv_sb, wv_d)):
                for j in range(8):
                    nc.scalar.dma_start(w_sb[:, j, :], w_dram[j * 128:(j + 1) * 128, :])
            wo_sb = wp.tile([128, D], F16)
            nc.scalar.dma_start(wo_sb, wo_d[:, :])

            # --- resident activations ---
            qt_sb = bigp.tile([128, T], F16)     # Q^T  [feat(2 heads x 64), tok]
            kt_sb = bigp.tile([128, T], F16)     # K^T
            ctxt_sb = bigp.tile([128, T], F16)   # normalized ctx^T (heads stacked)
            v_sb = bigp.tile([128, B, NKT, HPC * (HD + 1)], F16)  # [V_h|1] tiles
            ones_col = constp.tile([128, 1], F32)
            nc.gpsimd.memset(ones_col, 1.0)
            for h in range(HPC):
                c = h * (HD + 1) + HD
                nc.vector.tensor_copy(
                    v_sb[:, :, :, c:c + 1],
                    ones_col[:, None, None, :].broadcast_to([128, B, NKT, 1]),
                )

            # ---- interleaved stage A (projection) + stage B/C ----------
            # Engines execute their streams in emission order, so stage-A
            # work for later token spans is emitted between attention
            # chunks to keep every engine's stream in pipeline order.
            with (
                tc.tile_pool(name="xt", bufs=8) as xtp,
                tc.tile_pool(name="sbB", bufs=3) as sbB,
                tc.tile_pool(name="nrm", bufs=2) as nrm,
                tc.tile_pool(name="sbO", bufs=2) as sbO,
                tc.tile_pool(name="outp", bufs=2) as outp,
                tc.tile_pool(name="psA", bufs=1, space="PSUM") as psA,
                tc.tile_pool(name="psS", bufs=2, space="PSUM") as psS,
                tc.tile_pool(name="psC", bufs=2, space="PSUM") as psC,
                tc.tile_pool(name="psO", bufs=1, space="PSUM") as psO,
            ):
                def emit_a_dma(t0, w):
                    xt = xtp.tile([128, 8, CHUNK], F16, tag="xt")
                    if w <= 128:
                        for j in range(8):
                            nc.sync.dma_start_transpose(
                                xt[:, j, :w],
                                x_d[t0:t0 + w, j * 128:(j + 1) * 128],
                            )
                    else:
                        nc.sync.dma_start_transpose(
                            xt[:, :, :w], x_d[t0:t0 + w, :])
                    return xt

                def emit_a_proj(xt, t0, w):
                    for w_sb, dst in ((wq_sb, qt_sb), (wk_sb, kt_sb)):
                        pp = psA.tile([128, CHUNK], F32, tag="proj")
                        for j in range(8):
                            nc.tensor.matmul(
                                pp[:, :w], w_sb[:, j, :], xt[:, j, :w],
                                start=(j == 0), stop=(j == 7),
                            )
                        nc.vector.tensor_copy(dst[:, t0:t0 + w], pp[:, :w])
                    # V directly in natural [tok, feat] layout: xT.T @ Wv
                    b = t0 // S
                    for t in range(w // 128):
                        kt_idx = (t0 + t * 128 - b * S) // KT
                        pv = psA.tile([128, FPC], F32, tag="pv")
                        for j in range(8):
                            nc.tensor.matmul(
                                pv, xt[:, j, t * 128:(t + 1) * 128], wv_sb[:, j, :],
                                start=(j == 0), stop=(j == 7),
                            )
                        for h in range(HPC):
                            nc.vector.tensor_copy(
                                v_sb[:, b, kt_idx, h * (HD + 1):h * (HD + 1) + HD],
                                pv[:, h * HD:(h + 1) * HD],
                            )

                def emit_a(t0, w):
                    emit_a_proj(emit_a_dma(t0, w), t0, w)

                def emit_attn(ch):
                    b, qc = ch // (S // QC), ch % (S // QC)
                    for h in range(HPC):
                        pc = psC.tile([HD + 1, QC], F32, tag="ctx")
                        n_kt = (qc + 1) * (QC // KT)
                        for kt in range(n_kt):
                            diag = kt - qc * (QC // KT)
                            col_off = max(0, diag * KT)
                            n = QC - col_off
                            ps = psS.tile([128, QC], F32, tag="s")
                            nc.tensor.matmul(
                                ps[:, :n],
                                kt_sb[h * HD:(h + 1) * HD,
                                      b * S + kt * KT:b * S + (kt + 1) * KT],
                                qt_sb[h * HD:(h + 1) * HD,
                                      b * S + qc * QC + col_off:
                                      b * S + (qc + 1) * QC],
                                start=True, stop=True,
                            )
                            if diag >= 0:
                                nc.vector.tensor_add(
                                    ps[:, 0:KT], ps[:, 0:KT], cmask)
                            ex = sbB.tile([128, QC], F16, tag="exp")
                            nc.scalar.activation(
                                ex[:, :n], ps[:, :n],
                                mybir.ActivationFunctionType.Exp,
                                scale=0.125,
                            )
                            nc.tensor.matmul(
                                pc[:, col_off:QC],
                                v_sb[:, b, kt, h * (HD + 1):(h + 1) * (HD + 1)],
                                ex[:, :n],
                                start=(kt == 0), stop=(kt == n_kt - 1),
                            )
                        rrow = nrm.tile([1, QC], F32, tag="rrow")
                        nc.vector.reciprocal(rrow, pc[HD:HD + 1, :])
                        rec64 = nrm.tile([HD, QC], F32, tag="rec64")
                        nc.gpsimd.partition_broadcast(rec64, rrow)
                        nc.vector.tensor_mul(
                            ctxt_sb[h * HD:(h + 1) * HD,
                                    b * S + qc * QC:b * S + (qc + 1) * QC],
                            pc[0:HD, :], rec64,
                        )
                    # output projection for this chunk's 4 query tiles
                    for qt in range(QC // 128):
                        po = psO.tile([128, D], F32, tag="o")
                        row0 = ch * QC + qt * 128
                        for n in range(2):
                            nc.tensor.matmul(
                                po[:, n * 512:(n + 1) * 512],
                                ctxt_sb[:, row0:row0 + 128],
                                wo_sb[:, n * 512:(n + 1) * 512],
                                start=True, stop=True,
                            )
                        so = sbO.tile([128, D], F16, tag="so")
                        if qt % 2 == 0:
                            nc.scalar.copy(so, po)
                        else:
                            nc.vector.tensor_copy(so, po)
                        nc.gpsimd.dma_start(cc_in[ch][qt * 128:(qt + 1) * 128, :], so)
                    # reduce-scatter this chunk; overlaps later compute.
                    # Rank r receives rows [ch*QC + r*64, +64).
                    nc.gpsimd.collective_compute(
                        "ReduceScatter",
                        mybir.AluOpType.add,
                        replica_groups=[list(range(NCORES))],
                        ins=[cc_in[ch][:, :]],
                        outs=[cc_out[ch][:, :]],
                    )

                def emit_out(ch):
                    ot16 = outp.tile([QC // NCORES, D], F16, tag="ot16")
                    nc.sync.dma_start(ot16, cc_out[ch][:, :])
                    ot32 = outp.tile([QC // NCORES, D], F32, tag="ot32")
                    nc.vector.tensor_add(ot32, ot16, bo_bc[0:QC // NCORES, :])
                    nc.sync.dma_start(
                        out_d[ch * (QC // NCORES):(ch + 1) * (QC // NCORES), :],
                        ot32)

                # batch 0 light-first with A interleaved; batch 1
                # heavy-first so the tail chunks are light. The b1 xT
                # transpose DMAs are prefetched between b0 chunks; the
                # RS-dependent output stage is emitted last so it never
                # blocks the DMA queue.
                spans = [(0, 128), (128, 128), (256, 256), (512, 512),
                         (1024, 512), (1536, 512), (2048, 512),
                         (2560, 512), (3072, 512), (3584, 512)]
                xts = [emit_a_dma(t0, w) for t0, w in spans]
                emit_a_proj(xts[0], *spans[0])
                emit_a_proj(xts[1], *spans[1])
                emit_a_proj(xts[2], *spans[2])
                emit_attn(0)
                emit_a_proj(xts[3], *spans[3])
                emit_attn(1)
                emit_a_proj(xts[4], *spans[4])
                emit_attn(2)
                emit_a_proj(xts[5], *spans[5])
                emit_attn(3)
                emit_a_proj(xts[6], *spans[6])
                emit_a_proj(xts[7], *spans[7])
                emit_a_proj(xts[8], *spans[8])
                emit_a_proj(xts[9], *spans[9])
                emit_attn(7)
                emit_attn(6)
                emit_attn(5)
                emit_attn(4)
                for ch in [0, 1, 2, 3, 7, 6, 5, 4]:
                    emit_out(ch)

    nc.finalize()
    return nc


_NC_CACHE = []


def make_in_maps(x, Wq, Wk, Wv, Wo, bo):
    x = np.ascontiguousarray(np.asarray(x, dtype=np.float32)).reshape(T, D)
    x16 = x.astype(np.float16)
    Wq = np.asarray(Wq, dtype=np.float32).astype(np.float16)
    Wk = np.asarray(Wk, dtype=np.float32).astype(np.float16)
    Wv = np.asarray(Wv, dtype=np.float32).astype(np.float16)
    Wo = np.asarray(Wo, dtype=np.float32).astype(np.float16)
    bo = np.asarray(bo, dtype=np.float32).reshape(1, D)
    in_maps = []
    for c in range(NCORES):
        lo, hi = c * FPC, (c + 1) * FPC
        in_maps.append({
            "x": x16,
            "wq": np.ascontiguousarray(Wq[:, lo:hi]),
            "wk": np.ascontiguousarray(Wk[:, lo:hi]),
            "wv": np.ascontiguousarray(Wv[:, lo:hi]),
            "wo": np.ascontiguousarray(Wo[lo:hi, :]),
            "bo": bo,
        })
    return in_maps


def assemble_out(core_outs):
    # core r, chunk ch rows [ch*64, +64) = tokens [ch*512 + r*64, +64)
    stacked = np.stack(
        [np.asarray(o).reshape(NCH, QC // NCORES, D) for o in core_outs], axis=1
    )  # [ch, rank, 64, D]
    return stacked.reshape(B, S, D)


def kernel(x, Wq, Wk, Wv, Wo, bo):
    if not _NC_CACHE:
        _NC_CACHE.append(build_nc())
    nc = _NC_CACHE[0]
    in_maps = make_in_maps(x, Wq, Wk, Wv, Wo, bo)
    res = run_bass_kernel_spmd(nc, in_maps, core_ids=list(range(NCORES)))
    return assemble_out([r["out"] for r in res.results])
